# revision 1
# baseline (speedup 1.0000x reference)
"""GCN layer (X@W -> edge gather/scale -> segment-sum by dest -> +b -> relu)
as a Bass/Tile kernel on 8 Trainium2 NeuronCores.

Strategy (1D node partition, SPMD single program):
  - Nodes sharded 12500/core (destination shard).  Each core computes its
    XW shard with bf16 PE matmuls, then an AllGather replicates the full
    XW table into every core's DRAM as bf16 NODE PAIRS: table row p holds
    nodes (2p, 2p+1) = 128 bf16 = 256 bytes, the SWDGE gather element
    minimum.  Pairing halves gather bytes vs an f32 table and needs only
    2 idx windows (int16 covers 32768 pairs), cutting chunk padding.
  - Edges partitioned by destination shard, sorted by (dest block of 128,
    pair bucket).  Pair rows are fetched with dma_gather spread across 4
    SWDGE queues (4x the single-queue descriptor rate).
  - Per 128-edge chunk a parity-split selection matrix
    S[e, d + 128*par] = val[e] * (dest[e]==d) * (parity[e]==par)
    is built in ONE DVE tensor_scalar op (bf16 iota256 compare), then two
    PE matmuls accumulate psum[128 dests, 64] += S_par^T @ G_half.  A
    final ones/128 x bias matmul folds +b into PSUM; relu on eviction.

All chunk counts are padded to the max over cores so all 8 cores run the
same program (required for the collective / PJRT SPMD launch).
"""

import math
from contextlib import ExitStack

import numpy as np
from ml_dtypes import bfloat16

import concourse.bacc as bacc
import concourse.mybir as mybir
import concourse.tile as tile
from concourse.bass import _add_dep_helper
from concourse.bass_utils import run_bass_kernel_spmd

# Problem constants (hardcoded per contract; kernel.py must be self-contained).
N = 100000
E = 1600000
FIN = 256
FOUT = 64
NCORES = 8

P = 128                      # partitions / block size
SHARD = N // NCORES          # 12500 dest nodes per core
NBLK = math.ceil(SHARD / P)  # 98 dest blocks per core
SHARD_PAD = NBLK * P         # 12544 (X zero-padded rows)
PAIRS = SHARD_PAD // 2       # 6272 node pairs per core
TABLE_PAIRS = NCORES * PAIRS  # 50176 pair rows in the shared table
WIN = 32768                  # int16-addressable window (pair rows)
NWIN = 2
WIN_SPLIT = 25088            # pair < split -> window 0
WIN_BASE = (0, TABLE_PAIRS - WIN)  # idx = pair - base  (17408 for w1)
NPAR = 2                     # source-pair parities (chunks mostly parity-pure)
SB_BLOCKS = 4                # dest blocks per super-batch
NSB = math.ceil(NBLK / SB_BLOCKS)
KH = FIN // P                # 2 contraction halves in the GEMM
NQUEUES = 4                  # SWDGE queues for gathers (ucode max 4)
QDEPTH = 4                   # in-flight gather parts per queue
GMAXCH = 24                  # max chunks per gather part (~3072 descriptors)


def _build_plan(edge_row, edge_col, edge_vals):
    """Host-side edge partition/sort/pad.  Returns the uniform structure
    (shared across cores) + per-core staged arrays.

    Edges are segmented by (dest blk, pair window, source parity) so every
    128-edge chunk is parity-pure: its PE matmul reads a statically-known
    64-column half of the gathered pair rows."""
    core = edge_row // SHARD
    r_local = edge_row - core * SHARD
    blk = r_local // P
    dest_in_blk = r_local - blk * P
    src_core = edge_col // SHARD
    src_local = edge_col - src_core * SHARD
    parity = src_local % 2
    table_pair = src_core * PAIRS + src_local // 2
    win = (table_pair >= WIN_SPLIT).astype(np.int64)
    idx16 = (table_pair - np.asarray(WIN_BASE)[win]).astype(np.int16)
    dst = dest_in_blk.astype(np.float32)

    dst2 = (dest_in_blk + P * parity).astype(np.float32)

    # sort edges by (core, blk, win) with parity as the innermost key, so a
    # segment holds its par0 edges first and its par1 edges last
    order = np.lexsort((parity, win, blk, core))
    core_s = core[order]
    blk_s = blk[order]
    seg_s = win[order]
    par_s = parity[order]
    idx16_s = idx16[order]
    dst_s = dst2[order]
    val_s = edge_vals[order].astype(np.float32)

    # per-(core, blk, win) total and per-parity counts
    seg_key = (core_s * NBLK + blk_s) * NWIN + seg_s
    counts = np.bincount(seg_key, minlength=NCORES * NBLK * NWIN).reshape(
        NCORES, NBLK, NWIN
    )
    counts0 = np.bincount(seg_key[par_s == 0],
                          minlength=NCORES * NBLK * NWIN).reshape(
        NCORES, NBLK, NWIN
    )
    counts1 = counts - counts0
    # uniform capacity (in chunks of 128 edges) per (blk, win): max over cores
    chunks_bs = np.ceil(counts / P).astype(np.int64).max(axis=0)  # [NBLK, NWIN]
    # guarantee at least one chunk per block overall (needed so PSUM gets reset)
    assert chunks_bs.sum(axis=1).min() >= 1
    cap_bs = chunks_bs * P

    # ---- static layout ----
    # stream order: (sb, win, blk in sb, parity, chunk); gathers are per
    # (sb, win)
    sb_of_blk = np.arange(NBLK) // SB_BLOCKS
    slot_off = np.zeros((NBLK, NWIN), dtype=np.int64)
    sb_w_len = np.zeros((NSB, NWIN), dtype=np.int64)   # slots per (sb, win)
    for sb in range(NSB):
        blks = np.where(sb_of_blk == sb)[0]
        for w in range(NWIN):
            off = 0
            for bk in blks:
                slot_off[bk, w] = off
                off += cap_bs[bk, w]
            sb_w_len[sb, w] = off
    # global offsets: chunk columns and idx columns per (sb, win)
    chunk_col0 = np.zeros((NSB, NWIN), dtype=np.int64)
    idx_col0 = np.zeros((NSB, NWIN), dtype=np.int64)
    ccur = icur = 0
    for sb in range(NSB):
        for w in range(NWIN):
            chunk_col0[sb, w] = ccur
            idx_col0[sb, w] = icur
            ccur += sb_w_len[sb, w] // P
            icur += sb_w_len[sb, w] // 16
    CTOT = ccur   # total chunks per core
    ITOT = icur   # total idx columns per core

    # global slot index for every edge: par0 edges pack at the segment
    # front, par1 edges at the segment back (padding in the middle), so
    # leading/trailing chunks are parity-pure on every core
    first_of_seg = np.zeros(NCORES * NBLK * NWIN + 1, dtype=np.int64)
    np.cumsum(counts.reshape(-1), out=first_of_seg[1:])
    rank = np.arange(len(core_s)) - first_of_seg[seg_key]
    cap_e = cap_bs[blk_s, seg_s]
    n0_e = counts0[core_s, blk_s, seg_s]
    n1_e = counts1[core_s, blk_s, seg_s]
    within = np.where(par_s == 0, rank, cap_e - n1_e + (rank - n0_e))
    slot = (
        chunk_col0[sb_of_blk[blk_s], seg_s] * P
        + slot_off[blk_s, seg_s]
        + within
    )

    # ---- per-core staged arrays ----
    idx_streams = np.zeros((NCORES, CTOT * P), dtype=np.int16)
    dst_streams = np.zeros((NCORES, CTOT * P), dtype=np.float32)
    val_streams = np.zeros((NCORES, CTOT * P), dtype=np.float32)
    for c in range(NCORES):
        m = core_s == c
        idx_streams[c, slot[m]] = idx16_s[m]
        dst_streams[c, slot[m]] = dst_s[m]
        val_streams[c, slot[m]] = val_s[m]

    # dest/val DRAM layout [128, CTOT] f32: chunk j, partition p <- stream[j*128+p]
    dst_np = dst_streams.reshape(NCORES, CTOT, P).transpose(0, 2, 1).copy()
    val_np = val_streams.reshape(NCORES, CTOT, P).transpose(0, 2, 1).copy()

    # idx DRAM layout [128, ITOT] int16: within each (sb,w) segment of the
    # stream, idx i -> partition i%16 (replicated over the 8 groups of 16),
    # column i//16
    idx_np = np.zeros((NCORES, P, ITOT), dtype=np.int16)
    for sb in range(NSB):
        for w in range(NWIN):
            L = int(sb_w_len[sb, w])
            if L == 0:
                continue
            s0 = int(chunk_col0[sb, w]) * P
            i0 = int(idx_col0[sb, w])
            seg2 = idx_streams[:, s0:s0 + L].reshape(NCORES, L // 16, 16)
            seg2 = seg2.transpose(0, 2, 1)  # [NCORES, 16, L//16]
            idx_np[:, :, i0:i0 + L // 16] = np.tile(seg2, (1, 8, 1))

    # per-block chunk list: (win, j_local_in_gather, global_chunk_col, kind)
    # kind 0 = pure par0, 1 = pure par1, 2 = mixed/boundary
    n0_min = counts0.min(axis=0)   # [NBLK, NWIN]
    n1_min = counts1.min(axis=0)
    blk_chunks = []
    for bk in range(NBLK):
        sb = int(sb_of_blk[bk])
        lst = []
        for w in range(NWIN):
            nch = int(chunks_bs[bk, w])
            j0 = int(slot_off[bk, w]) // P
            c0 = int(chunk_col0[sb, w]) + j0
            pure0_end = int(n0_min[bk, w]) // P
            pure1_start = -(-(int(cap_bs[bk, w]) - int(n1_min[bk, w])) // P)
            for k in range(nch):
                kind = 0 if k < pure0_end else (1 if k >= pure1_start else 2)
                lst.append((w, j0 + k, c0 + k, kind))
        blk_chunks.append(lst)

    struct = dict(
        chunks_bs=chunks_bs, sb_w_len=sb_w_len, chunk_col0=chunk_col0,
        idx_col0=idx_col0, CTOT=CTOT, ITOT=ITOT, blk_chunks=blk_chunks,
        sb_of_blk=sb_of_blk,
    )
    return struct, idx_np, dst_np, val_np


_NO_SPLIT = ("InstEventSemaphore", "InstDrain", "InstCollectiveCompute",
             "InstCall", "InstUnconditionalBranch", "InstConditionalBranch")


def _split_excess_waits(nc):
    """Deterministic post-pass: TRN2 instructions tolerate very few sync
    waits (walrus rejects with 'Too many sync wait commands'; Bacc's own
    generate_event_semaphores pass misses cases).  Move all but one
    semaphore wait of every ordinary instruction onto wait-only
    InstEventSemaphore instructions inserted just before it on the same
    engine (engine program order then gates the original instruction)."""
    import concourse.mybir as mybir

    for blk in nc.main_func.blocks:
        out = []
        for ins in blk.instructions:
            si = ins.sync_info
            tn = type(ins).__name__
            if si is None or tn in _NO_SPLIT or len(si.on_wait) <= 1:
                out.append(ins)
                continue
            waits = list(si.on_wait)
            keep, excess = waits[:1], waits[1:]
            while excess:
                batch, excess = excess[:2], excess[2:]
                ev = mybir.InstEventSemaphore(
                    name=nc.get_next_instruction_name(), ins=[], outs=[])
                ev.engine = ins.engine
                ev.sync_info = mybir.SyncInfo(on_wait=batch, on_update=[])
                out.append(ev)
            ins.sync_info = mybir.SyncInfo(
                on_wait=keep, on_update=list(si.on_update))
            out.append(ins)
        blk.instructions[:] = out


def _build_nc(struct, variant="full"):
    # variant: "full" | "p12" (GEMM+collective, dummy out) |
    #          "p12g" (+ gathers, dummy out) | "p1" (GEMM only) |
    #          "nop" (launch overhead floor) | "ag" (collective only) |
    #          "go" (gathers only, garbage table)
    # "nos" = full but constant S (no per-chunk DVE); "nomm" = full but no
    # per-chunk matmuls.  Both are wrong-result timing ablations.
    # A "xN" suffix (e.g. "full x8") repeats the kernel body N times inside
    # one NEFF so device time dominates the per-exec dispatch floor.
    reps = 1
    if " x" in variant:
        variant, reps_s = variant.split(" x")
        reps = int(reps_s)
    st = struct
    CTOT, ITOT = st["CTOT"], st["ITOT"]
    nc = bacc.Bacc("TRN2", target_bir_lowering=False, debug=False,
                   num_devices=NCORES, num_swdge_queues=NQUEUES)
    f32 = mybir.dt.float32
    bf16 = mybir.dt.bfloat16
    i16 = mybir.dt.int16

    xt_sh = nc.dram_tensor("xt_sh", [FIN, SHARD_PAD], bf16, kind="ExternalInput")
    w_in = nc.dram_tensor("w_in", [FIN, FOUT], bf16, kind="ExternalInput")
    b_rep = nc.dram_tensor("b_rep", [P, FOUT], bf16, kind="ExternalInput")
    iota_in = nc.dram_tensor("iota_in", [P, 2 * P], bf16, kind="ExternalInput")
    idx_in = nc.dram_tensor("idx_in", [P, ITOT], i16, kind="ExternalInput")
    dest_in = nc.dram_tensor("dest_in", [P, CTOT], f32, kind="ExternalInput")
    val_in = nc.dram_tensor("val_in", [P, CTOT], f32, kind="ExternalInput")

    # xw_sh row-major [node, feat] bf16 == [pair, 128] bf16 pair rows
    xw_sh = nc.dram_tensor("xw_sh", [SHARD_PAD, FOUT], bf16, kind="Internal")
    table = nc.dram_tensor("table", [TABLE_PAIRS, 2 * FOUT], bf16,
                           kind="Internal", addr_space="Shared")
    out_sh = nc.dram_tensor("out_sh", [SHARD_PAD, FOUT], f32,
                            kind="ExternalOutput")

    with tile.TileContext(nc) as tc, ExitStack() as ctx:
        consts = ctx.enter_context(tc.tile_pool(name="consts", bufs=1))
        gpool = ctx.enter_context(tc.tile_pool(name="gpool", bufs=3))
        spool = ctx.enter_context(tc.tile_pool(name="spool", bufs=16))
        s2pool = ctx.enter_context(tc.tile_pool(name="s2pool", bufs=8))
        opool = ctx.enter_context(tc.tile_pool(name="opool", bufs=4))
        xpool = ctx.enter_context(tc.tile_pool(name="xpool", bufs=3))
        pmpool = ctx.enter_context(
            tc.tile_pool(name="pmpool", bufs=2, space="PSUM"))
        popool = ctx.enter_context(
            tc.tile_pool(name="popool", bufs=6, space="PSUM"))

        do_gemm = variant in ("full", "nos", "nomm", "p12", "p12g", "p1")
        do_ag = variant in ("full", "nos", "nomm", "p12", "p12g", "ag")
        do_gather = variant in ("full", "nos", "nomm", "p12g", "go")
        do_compute = variant in ("full", "nos", "nomm")
        do_sbuild = variant != "nos"
        do_mm = variant != "nomm"

        brep_t = consts.tile([P, FOUT], bf16)
        nc.sync.dma_start(out=brep_t[:], in_=b_rep[:])
        if do_compute:
            iota_t = consts.tile([P, 2 * P], bf16)
            nc.sync.dma_start(out=iota_t[:], in_=iota_in[:])
            ones_t = consts.tile([P, P], bf16, tag="ones")
            nc.vector.memset(ones_t[:], 1.0 / P)
            dst_all = consts.tile([P, CTOT], f32, tag="dstall")
            nc.sync.dma_start(out=dst_all[:], in_=dest_in[:])
            vl_all = consts.tile([P, CTOT], f32, tag="vlall")
            nc.sync.dma_start(out=vl_all[:], in_=val_in[:])
        if do_gather:
            idx_all = consts.tile([P, ITOT], i16, tag="idxall")
            nc.sync.dma_start(out=idx_all[:], in_=idx_in[:])
        if do_gemm:
            w_t = []
            for h in range(KH):
                wt = consts.tile([P, FOUT], bf16, tag=f"w{h}")
                nc.sync.dma_start(out=wt[:], in_=w_in[h * P:(h + 1) * P, :])
                w_t.append(wt)

        # ---------------- phase 1: GEMM shard ----------------
        # X^T comes pre-transposed (bf16) from the host, so lhsT tiles are
        # plain big strided loads and PE needs no transposes.
        GRP = 896 if SHARD_PAD % 896 == 0 else SHARD_PAD
        assert SHARD_PAD % GRP == 0 and GRP % P == 0
        gather_insts = {q: [] for q in range(NQUEUES)}
        gq = [0]
        for rep in range(reps):
          for g in (range(SHARD_PAD // GRP) if do_gemm else ()):
              xts = []
              for h in range(KH):
                  xt = xpool.tile([P, GRP], bf16, tag=f"xt{h}")
                  nc.sync.dma_start(
                      out=xt[:], in_=xt_sh[h * P:(h + 1) * P,
                                           g * GRP:(g + 1) * GRP])
                  xts.append(xt)
              for c in range(GRP // P):
                  bk = g * (GRP // P) + c
                  mm = pmpool.tile([P, FOUT], f32, tag="mm")
                  for h in range(KH):
                      nc.tensor.matmul(
                          out=mm[:], lhsT=xts[h][:, c * P:(c + 1) * P],
                          rhs=w_t[h][:], start=(h == 0), stop=(h == KH - 1))
                  om = opool.tile([P, FOUT], bf16, tag="om")
                  nc.vector.tensor_copy(out=om[:], in_=mm[:])
                  nc.sync.dma_start(
                      out=xw_sh[bk * P:(bk + 1) * P, :], in_=om[:])

          # ---------------- phase 2: AllGather the XW pair table ----------------
          if do_ag:
              nc.gpsimd.collective_compute(
                  kind="AllGather", op=mybir.AluOpType.bypass,
                  replica_groups=[list(range(NCORES))],
                  ins=[xw_sh[:]], outs=[table[:]],
              )

          # ---------------- phase 3: gather + segment-sum ----------------
          chunks_bs = st["chunks_bs"]
          sb_w_len = st["sb_w_len"]
          chunk_col0 = st["chunk_col0"]
          idx_col0 = st["idx_col0"]
          blk_chunks = st["blk_chunks"]
          sb_of_blk = st["sb_of_blk"]


          if not do_compute:
              ob = opool.tile([P, FOUT], f32, tag="ob")
              obb = opool.tile([P, FOUT], bf16, tag="obb")
              if do_ag:
                  nc.sync.dma_start(out=obb[:], in_=table[:P, :FOUT])
              elif do_gemm:
                  nc.sync.dma_start(out=obb[:], in_=xw_sh[:P, :FOUT])
              else:
                  nc.sync.dma_start(out=obb[:], in_=b_rep[:])
              nc.vector.tensor_copy(out=ob[:], in_=obb[:])
              nc.sync.dma_start(out=out_sh[:P, :], in_=ob[:])

          for sb in (range(NSB) if do_gather else ()):
              blks = [bk for bk in range(NBLK) if sb_of_blk[bk] == sb]

              gts = [None] * NWIN
              for w in range(NWIN):
                  L = int(sb_w_len[sb, w])
                  if L == 0:
                      continue
                  nch = L // P
                  icol = int(idx_col0[sb, w])
                  gt = gpool.tile([P, nch * 2 * FOUT], bf16, tag=f"g{w}")
                  r_lo = WIN_BASE[w]
                  r_hi = r_lo + WIN
                  # split into parts of <=GMAXCH chunks; parts round-robin
                  # the SWDGE queues so one (sb,b) gather engages all rings.
                  nparts = max(1, math.ceil(nch / GMAXCH))
                  psz = math.ceil(nch / nparts)
                  for c0 in range(0, nch, psz):
                      c1 = min(c0 + psz, nch)
                      q = gq[0] % NQUEUES
                      gq[0] += 1
                      gi = nc.gpsimd.dma_gather(
                          out_ap=gt[:, c0 * 2 * FOUT:c1 * 2 * FOUT].rearrange(
                              "p (c f) -> p c f", f=2 * FOUT),
                          in_ap=table[r_lo:r_hi, :],
                          idxs_ap=idx_all[:, icol + c0 * 8:icol + c1 * 8],
                          num_idxs=(c1 - c0) * P,
                          num_idxs_reg=(c1 - c0) * P,
                          elem_size=2 * FOUT,
                          single_packet=False,
                          queue_num=q,
                      )
                      if len(gather_insts[q]) >= QDEPTH:
                          _add_dep_helper(gi.ins, gather_insts[q][-QDEPTH],
                                          sync=True,
                                          reason="swdge ring throttle")
                      gather_insts[q].append(gi.ins)
                  gts[w] = gt

              if not do_compute:
                  continue

              for bk in blks:
                  po = popool.tile([P, FOUT], f32, tag="po")
                  lst = blk_chunks[bk]
                  started = False
                  for k, (w, j, gcol, kind) in enumerate(lst):
                      g0 = j * 2 * FOUT
                      if kind < 2:
                          if do_sbuild:
                              s_t = spool.tile([P, P], bf16, tag="s")
                              nc.vector.tensor_scalar(
                                  out=s_t[:],
                                  in0=iota_t[:, kind * P:kind * P + P],
                                  scalar1=dst_all[:, gcol:gcol + 1],
                                  scalar2=vl_all[:, gcol:gcol + 1],
                                  op0=mybir.AluOpType.is_equal,
                                  op1=mybir.AluOpType.mult,
                              )
                          else:
                              s_t = ones_t
                          if do_mm:
                              nc.tensor.matmul(
                                  out=po[:], lhsT=s_t[:],
                                  rhs=gts[w][:, g0 + kind * FOUT:
                                             g0 + (kind + 1) * FOUT],
                                  start=not started, stop=False,
                              )
                              started = True
                      else:
                          if do_sbuild:
                              s2_t = s2pool.tile([P, 2 * P], bf16, tag="s2")
                              nc.vector.tensor_scalar(
                                  out=s2_t[:], in0=iota_t[:],
                                  scalar1=dst_all[:, gcol:gcol + 1],
                                  scalar2=vl_all[:, gcol:gcol + 1],
                                  op0=mybir.AluOpType.is_equal,
                                  op1=mybir.AluOpType.mult,
                              )
                          else:
                              s2_t = iota_t
                          if do_mm:
                              nc.tensor.matmul(
                                  out=po[:], lhsT=s2_t[:, :P],
                                  rhs=gts[w][:, g0:g0 + FOUT],
                                  start=not started, stop=False,
                              )
                              nc.tensor.matmul(
                                  out=po[:], lhsT=s2_t[:, P:],
                                  rhs=gts[w][:, g0 + FOUT:g0 + 2 * FOUT],
                                  start=False, stop=False,
                              )
                              started = True
                  # fold +bias into PSUM: ones/128 selector x replicated bias
                  nc.tensor.matmul(
                      out=po[:], lhsT=ones_t[:], rhs=brep_t[:],
                      start=not started, stop=True,
                  )
                  ob = opool.tile([P, FOUT], f32, tag="ob")
                  nc.scalar.activation(
                      out=ob[:], in_=po[:],
                      func=mybir.ActivationFunctionType.Relu)
                  nc.sync.dma_start(
                      out=out_sh[bk * P:(bk + 1) * P, :], in_=ob[:])

    nc.compile()
    _split_excess_waits(nc)
    return nc


def _prepare(X, edge_row, edge_col, edge_vals, W, b):
    """Build the compiled Bass program + per-core input maps."""
    X = np.asarray(X, dtype=np.float32)
    edge_row = np.asarray(edge_row, dtype=np.int64)
    edge_col = np.asarray(edge_col, dtype=np.int64)
    edge_vals = np.asarray(edge_vals, dtype=np.float32)
    W = np.asarray(W, dtype=np.float32)
    b = np.asarray(b, dtype=np.float32)

    struct, idx_np, dst2_np, val_np = _build_plan(edge_row, edge_col, edge_vals)
    nc = _build_nc(struct)

    b_rep = np.tile(b[None, :], (P, 1)).astype(bfloat16)
    iota = np.tile(np.arange(2 * P, dtype=np.float32)[None, :], (P, 1))
    iota = iota.astype(bfloat16)

    in_maps = []
    for c in range(NCORES):
        xt_pad = np.zeros((FIN, SHARD_PAD), dtype=bfloat16)
        xt_pad[:, :SHARD] = X[c * SHARD:(c + 1) * SHARD].T.astype(bfloat16)
        in_maps.append({
            "xt_sh": xt_pad, "w_in": W.astype(bfloat16), "b_rep": b_rep,
            "iota_in": iota, "idx_in": idx_np[c], "dest_in": dst2_np[c],
            "val_in": val_np[c],
        })
    return nc, in_maps


def _assemble(results):
    return np.concatenate(
        [results[c]["out_sh"][:SHARD] for c in range(NCORES)], axis=0)


def kernel(X, edge_row, edge_col, edge_vals, W, b):
    nc, in_maps = _prepare(X, edge_row, edge_col, edge_vals, W, b)
    res = run_bass_kernel_spmd(nc, in_maps, core_ids=list(range(NCORES)))
    return _assemble(res.results)



# revision 3
# speedup vs baseline: 2.1670x; 2.1670x over previous
"""GCN layer (X@W -> edge gather/scale -> segment-sum by dest -> +b -> relu)
as a Bass/Tile kernel on 8 Trainium2 NeuronCores — v3 (quad table + host-S).

Strategy (1D node partition, SPMD single program):
  - Nodes relabeled on host (degree-balanced snake deal) so every
    (core, 128-dest block) holds ~equal edge counts -> chunk padding is
    near-zero.  Nodes sharded 12500/core.
  - Each core computes its XW shard with bf16 PE matmuls; an AllGather
    replicates the full XW table as bf16 QUAD rows: table row q holds
    nodes 4q..4q+3 = 256 bf16 = 512B.  512B SWDGE elements avoid the
    <512B read-modify-write DMA penalty, and 25088 quad rows fit one
    int16 index window (no window split, halving chunk padding).
  - Edges partitioned by dest core, bucketed by (dest blk, src qpos),
    qpos groups packed back-to-back per core.  Each 128-edge chunk
    gathers its 128 source quad rows via dma_gather on 4 SWDGE queues.
  - The selection matrix S[e, d] = val[e] * (dest[e]==d) is built ON THE
    HOST (bf16) and streamed from DRAM - measurement shows DVE/Act ops
    serialize with SWDGE gather traffic on this hardware, so the S build
    must not touch the engines.  Per chunk, one PE matmul per source
    qpos present accumulates psum[128 dests, 64] += S_q^T @ G[:, q].
    A ones/128 x bias matmul folds +b into PSUM; relu on eviction.

All chunk counts are padded to the max over cores so all 8 cores run the
same program (required for the collective / PJRT SPMD launch).
"""

import math
from contextlib import ExitStack

import numpy as np
from ml_dtypes import bfloat16

import concourse.bacc as bacc
import concourse.mybir as mybir
import concourse.tile as tile
from concourse.bass import _add_dep_helper
from concourse.bass_utils import run_bass_kernel_spmd

# Problem constants (hardcoded per contract; kernel must be self-contained).
N = 100000
E = 1600000
FIN = 256
FOUT = 64
NCORES = 8

P = 128                       # partitions / block size
SHARD = N // NCORES           # 12500 dest nodes per core
NBLK = math.ceil(SHARD / P)   # 98 dest blocks per core
SHARD_PAD = NBLK * P          # 12544 (X zero-padded rows)
NQ = 4                        # nodes per table quad row
QUADS = SHARD_PAD // NQ       # 3136 quad rows per core
TABLE_QUADS = NCORES * QUADS  # 25088 quad rows (< 32768: one int16 window)
SB_BLOCKS = 2                 # dest blocks per super-batch
NSB = math.ceil(NBLK / SB_BLOCKS)


def set_sb(n):
    """Reconfigure super-batch width (affects plan + program)."""
    global SB_BLOCKS, NSB
    SB_BLOCKS = n
    NSB = math.ceil(NBLK / SB_BLOCKS)
KH = FIN // P                 # 2 contraction halves in the GEMM
NQUEUES = 4                   # SWDGE queues for gathers (ucode max 4)
QDEPTH = 64                   # in-flight gather parts per queue before a
                              # throttle dep (33 parts/queue in practice, so
                              # effectively unthrottled; SWDGE ring holds 128)
GMAXCH = 12                   # max chunks per gather part


def _balance_perm(edge_row):
    """Relabel nodes so per-(core, blk) dest-edge counts are balanced.

    Snake-deals nodes in degree-desc order into the 784 (core, blk) bins
    (mirroring the id->(core,blk) map incl. the 84-slot tail block per
    core).  Returns perm_old_of_new [N]: new id -> old node."""
    deg = np.bincount(edge_row, minlength=N).astype(np.int64)
    order = np.argsort(-deg, kind="stable")
    nbins = NCORES * NBLK
    cap = np.full(nbins, P, dtype=np.int64)
    cap[NBLK - 1::NBLK] = SHARD - (NBLK - 1) * P
    bin_base = (np.arange(nbins) // NBLK) * SHARD + (np.arange(nbins) % NBLK) * P
    perm = np.empty(N, dtype=np.int64)
    fill = np.zeros(nbins, dtype=np.int64)
    pos = 0
    fwd = True
    bins_seq = np.arange(nbins)
    while pos < N:
        avail = bins_seq[fill < cap]
        if not fwd:
            avail = avail[::-1]
        take = min(len(avail), N - pos)
        sel = avail[:take]
        perm[bin_base[sel] + fill[sel]] = order[pos:pos + take]
        fill[sel] += 1
        pos += take
        fwd = not fwd
    return perm


def _build_plan(edge_row, edge_col, edge_vals):
    """Host-side edge partition/sort/pad.

    Edges are bucketed (core, blk) with qpos groups packed back-to-back
    per core; chunk capacity per blk = ceil(max-core count / 128).  Per
    chunk the UNION over cores of source qpos groups present decides the
    matmul list; absent groups simply have all-zero S columns."""
    core = edge_row // SHARD
    r_local = edge_row - core * SHARD
    blk = r_local // P
    dst = (r_local - blk * P).astype(np.int64)
    src_core = edge_col // SHARD
    src_local = edge_col - src_core * SHARD
    qpos = src_local % NQ
    quad = src_core * QUADS + src_local // NQ     # < 25088, int16-safe

    order = np.lexsort((qpos, blk, core))
    core_s = core[order]
    blk_s = blk[order]
    qpos_s = qpos[order]
    quad_s = quad[order]
    dst_s = dst[order]
    val_s = edge_vals[order].astype(np.float32)

    # counts per (core, blk, qpos) and per (core, blk)
    key_cbq = (core_s * NBLK + blk_s) * NQ + qpos_s
    counts_cbq = np.bincount(key_cbq, minlength=NCORES * NBLK * NQ).reshape(
        NCORES, NBLK, NQ)
    counts_cb = counts_cbq.sum(axis=2)
    pfx_cbq = np.zeros((NCORES, NBLK, NQ + 1), dtype=np.int64)
    np.cumsum(counts_cbq, axis=2, out=pfx_cbq[:, :, 1:])

    # per-blk chunk capacity = ceil(max-core total / 128)
    nch_b = np.ceil(counts_cb.max(axis=0) / P).astype(np.int64)  # [NBLK]
    assert nch_b.min() >= 1
    CTOT = int(nch_b.sum())
    ITOT = CTOT * (P // 16)

    sb_of_blk = np.arange(NBLK) // SB_BLOCKS
    # global chunk column of each blk's first chunk (sb-major walk order)
    blk_c0 = np.zeros(NBLK, dtype=np.int64)
    cur = 0
    for sb in range(NSB):
        for bk in np.where(sb_of_blk == sb)[0]:
            blk_c0[bk] = cur
            cur += nch_b[bk]
    assert cur == CTOT
    sb_len = np.zeros(NSB, dtype=np.int64)     # chunks per sb
    sb_c0 = np.zeros(NSB + 1, dtype=np.int64)  # first chunk col of sb
    for sb in range(NSB):
        bks = np.where(sb_of_blk == sb)[0]
        sb_c0[sb] = blk_c0[bks[0]]
        sb_len[sb] = nch_b[bks].sum()
    sb_c0[NSB] = CTOT

    # per-chunk qpos union over cores; blk_chunks[bk] = [(gcol, qlist)]
    blk_chunks = []
    for bk in range(NBLK):
        lst = []
        for k in range(int(nch_b[bk])):
            lo, hi = k * P, (k + 1) * P
            ql = []
            for g in range(NQ):
                lo_g = pfx_cbq[:, bk, g]
                hi_g = pfx_cbq[:, bk, g + 1]
                if np.any((lo_g < hi) & (hi_g > lo)):
                    ql.append(g)
            lst.append((int(blk_c0[bk]) + k, ql))
        blk_chunks.append(lst)

    # S column layout: walk sb -> blk -> chunk -> qpos in qlist
    scol = {}
    s_col0 = np.zeros(NSB + 1, dtype=np.int64)
    cur = 0
    for sb in range(NSB):
        s_col0[sb] = cur
        for bk in np.where(sb_of_blk == sb)[0]:
            for (gcol, ql) in blk_chunks[bk]:
                for g in ql:
                    scol[(gcol, g)] = cur
                    cur += P
    s_col0[NSB] = cur
    STOT = cur

    # per-edge global slot: qpos groups packed back-to-back per (core, blk)
    first_cbq = np.zeros(NCORES * NBLK * NQ + 1, dtype=np.int64)
    np.cumsum(counts_cbq.reshape(-1), out=first_cbq[1:])
    rank = np.arange(len(core_s)) - first_cbq[key_cbq]
    within = pfx_cbq[core_s, blk_s, qpos_s] + rank
    slot = blk_c0[blk_s] * P + within
    gcol_e = slot // P
    eslot_e = slot - gcol_e * P

    # staged arrays: idx [128, ITOT] int16 wrapped per sb; S [128, STOT] bf16
    idx_streams = np.zeros((NCORES, CTOT * P), dtype=np.int16)
    for c in range(NCORES):
        m = core_s == c
        idx_streams[c, slot[m]] = quad_s[m].astype(np.int16)
    idx_np = np.zeros((NCORES, P, ITOT), dtype=np.int16)
    for sb in range(NSB):
        L = int(sb_len[sb]) * P
        if L == 0:
            continue
        s0 = int(sb_c0[sb]) * P
        i0 = int(sb_c0[sb]) * (P // 16)
        seg = idx_streams[:, s0:s0 + L].reshape(NCORES, L // 16, 16)
        seg = seg.transpose(0, 2, 1)
        idx_np[:, :, i0:i0 + L // 16] = np.tile(seg, (1, 8, 1))

    scol_arr = np.full((CTOT, NQ), -1, dtype=np.int64)
    for (gcol, g), c0 in scol.items():
        scol_arr[gcol, g] = c0
    col_e = scol_arr[gcol_e, qpos_s] + dst_s
    assert (scol_arr[gcol_e, qpos_s] >= 0).all()
    s_np = np.zeros((NCORES, P, STOT), dtype=bfloat16)
    for c in range(NCORES):
        m = core_s == c
        s_np[c, eslot_e[m], col_e[m]] = val_s[m].astype(bfloat16)

    struct = dict(
        nch_b=nch_b, sb_of_blk=sb_of_blk, blk_c0=blk_c0, sb_len=sb_len,
        sb_c0=sb_c0, blk_chunks=blk_chunks, s_col0=s_col0, scol=scol,
        CTOT=CTOT, ITOT=ITOT, STOT=STOT,
    )
    return struct, idx_np, s_np


_NO_SPLIT = ("InstEventSemaphore", "InstDrain", "InstCollectiveCompute",
             "InstCall", "InstUnconditionalBranch", "InstConditionalBranch")


def _split_excess_waits(nc):
    """Deterministic post-pass: TRN2 instructions tolerate very few sync
    waits (walrus rejects with 'Too many sync wait commands').  Move all
    but one semaphore wait of every ordinary instruction onto wait-only
    InstEventSemaphore instructions inserted just before it on the same
    engine."""
    for blk in nc.main_func.blocks:
        out = []
        for ins in blk.instructions:
            si = ins.sync_info
            tn = type(ins).__name__
            if si is None or tn in _NO_SPLIT or len(si.on_wait) <= 1:
                out.append(ins)
                continue
            waits = list(si.on_wait)
            keep, excess = waits[:1], waits[1:]
            while excess:
                batch, excess = excess[:2], excess[2:]
                ev = mybir.InstEventSemaphore(
                    name=nc.get_next_instruction_name(), ins=[], outs=[])
                ev.engine = ins.engine
                ev.sync_info = mybir.SyncInfo(on_wait=batch, on_update=[])
                out.append(ev)
            ins.sync_info = mybir.SyncInfo(
                on_wait=keep, on_update=list(si.on_update))
            out.append(ins)
        blk.instructions[:] = out


def _build_nc(struct, variant="full", cfg=None):
    # variant: "full" | "go" (gathers only) | "hsgs" (gathers + S stream)
    #          | "p1" (GEMM) | "p12" (GEMM+AG) | "nop"
    # "xN" suffix repeats the body N times inside one NEFF (timing).
    cfg = {**dict(qdepth=QDEPTH, gmaxch=GMAXCH, single_packet=False),
           **(cfg or {})}
    reps = 1
    if " x" in variant:
        variant, reps_s = variant.split(" x")
        reps = int(reps_s)
    st = struct
    CTOT, ITOT, STOT = st["CTOT"], st["ITOT"], st["STOT"]
    nc = bacc.Bacc("TRN2", target_bir_lowering=False, debug=False,
                   num_devices=NCORES, num_swdge_queues=NQUEUES)
    f32 = mybir.dt.float32
    bf16 = mybir.dt.bfloat16
    i16 = mybir.dt.int16

    xt_sh = nc.dram_tensor("xt_sh", [FIN, SHARD_PAD], bf16, kind="ExternalInput")
    w_in = nc.dram_tensor("w_in", [FIN, FOUT], bf16, kind="ExternalInput")
    b_rep = nc.dram_tensor("b_rep", [P, FOUT], bf16, kind="ExternalInput")
    idx_in = nc.dram_tensor("idx_in", [P, ITOT], i16, kind="ExternalInput")
    s_in = nc.dram_tensor("s_in", [P, STOT], bf16, kind="ExternalInput")

    xw_sh = nc.dram_tensor("xw_sh", [SHARD_PAD, FOUT], bf16, kind="Internal")
    table = nc.dram_tensor("table", [TABLE_QUADS, NQ * FOUT], bf16,
                           kind="Internal", addr_space="Shared")
    out_sh = nc.dram_tensor("out_sh", [SHARD_PAD, FOUT], f32,
                            kind="ExternalOutput")

    do_gemm = variant in ("full", "p1", "p12", "p12g")
    do_ag = variant in ("full", "p12", "p12g", "ag")
    do_gather = variant in ("full", "go", "hsgs", "p12g")
    do_stream = variant in ("full", "hsgs")
    do_compute = variant == "full"

    with tile.TileContext(nc) as tc, ExitStack() as ctx:
        consts = ctx.enter_context(tc.tile_pool(name="consts", bufs=1))
        gpool = ctx.enter_context(tc.tile_pool(name="gpool", bufs=3))
        hspool = ctx.enter_context(tc.tile_pool(name="hspool", bufs=3))
        opool = ctx.enter_context(tc.tile_pool(name="opool", bufs=4))
        xpool = ctx.enter_context(tc.tile_pool(name="xpool", bufs=3))
        pmpool = ctx.enter_context(
            tc.tile_pool(name="pmpool", bufs=2, space="PSUM"))
        popool = ctx.enter_context(
            tc.tile_pool(name="popool", bufs=6, space="PSUM"))

        brep_t = consts.tile([P, FOUT], bf16)
        nc.sync.dma_start(out=brep_t[:], in_=b_rep[:])
        if do_compute:
            ones_t = consts.tile([P, P], bf16, tag="ones")
            nc.vector.memset(ones_t[:], 1.0 / P)
        if do_gather:
            idx_all = consts.tile([P, ITOT], i16, tag="idxall")
            nc.sync.dma_start(out=idx_all[:], in_=idx_in[:])
        if do_gemm:
            w_t = []
            for h in range(KH):
                wt = consts.tile([P, FOUT], bf16, tag=f"w{h}")
                nc.sync.dma_start(out=wt[:], in_=w_in[h * P:(h + 1) * P, :])
                w_t.append(wt)

        QD, GM, SP = cfg["qdepth"], cfg["gmaxch"], cfg["single_packet"]
        gather_insts = {q: [] for q in range(NQUEUES)}
        gq = [0]

        for rep in range(reps):
          # ---------------- phase 1: GEMM shard ----------------
          GRP = 896 if SHARD_PAD % 896 == 0 else SHARD_PAD
          for g in (range(SHARD_PAD // GRP) if do_gemm else ()):
              xts = []
              for h in range(KH):
                  xt = xpool.tile([P, GRP], bf16, tag=f"xt{h}")
                  nc.sync.dma_start(
                      out=xt[:], in_=xt_sh[h * P:(h + 1) * P,
                                           g * GRP:(g + 1) * GRP])
                  xts.append(xt)
              for c in range(GRP // P):
                  bk = g * (GRP // P) + c
                  mm = pmpool.tile([P, FOUT], f32, tag="mm")
                  for h in range(KH):
                      nc.tensor.matmul(
                          out=mm[:], lhsT=xts[h][:, c * P:(c + 1) * P],
                          rhs=w_t[h][:], start=(h == 0), stop=(h == KH - 1))
                  om = opool.tile([P, FOUT], bf16, tag="om")
                  nc.vector.tensor_copy(out=om[:], in_=mm[:])
                  nc.sync.dma_start(
                      out=xw_sh[bk * P:(bk + 1) * P, :], in_=om[:])

          # ---------------- phase 2: AllGather the XW quad table ------------
          if do_ag:
              nc.gpsimd.collective_compute(
                  kind="AllGather", op=mybir.AluOpType.bypass,
                  replica_groups=[list(range(NCORES))],
                  ins=[xw_sh[:]], outs=[table[:]],
              )

          # ---------------- phase 3: gather + segment-sum ----------------
          if not do_compute:
              ob = opool.tile([P, FOUT], f32, tag="ob")
              obb = opool.tile([P, FOUT], bf16, tag="obb")
              if do_ag:
                  nc.sync.dma_start(out=obb[:], in_=table[:P, :FOUT])
              elif do_gemm:
                  nc.sync.dma_start(out=obb[:], in_=xw_sh[:P, :FOUT])
              else:
                  nc.sync.dma_start(out=obb[:], in_=b_rep[:])
              nc.vector.tensor_copy(out=ob[:], in_=obb[:])
              nc.sync.dma_start(out=out_sh[:P, :], in_=ob[:])

          sb_of_blk = st["sb_of_blk"]
          for sb in (range(NSB) if (do_gather or do_stream) else ()):
              blks = [bk for bk in range(NBLK) if sb_of_blk[bk] == sb]
              nch = int(st["sb_len"][sb])
              c0g = int(st["sb_c0"][sb])

              if do_stream:
                  sc0 = int(st["s_col0"][sb])
                  sc1 = int(st["s_col0"][sb + 1])
                  s_sb = hspool.tile([P, sc1 - sc0], bf16, tag="ssb")
                  nc.sync.dma_start(out=s_sb[:], in_=s_in[:, sc0:sc1])

              gt = None
              if do_gather:
                  icol = c0g * (P // 16)
                  gt = gpool.tile([P, nch * NQ * FOUT], bf16, tag="g")
                  nparts = max(1, math.ceil(nch / GM))
                  psz = math.ceil(nch / nparts)
                  for c0 in range(0, nch, psz):
                      c1 = min(c0 + psz, nch)
                      q = gq[0] % NQUEUES
                      gq[0] += 1
                      gi = nc.gpsimd.dma_gather(
                          out_ap=gt[:, c0 * NQ * FOUT:c1 * NQ * FOUT].rearrange(
                              "p (c f) -> p c f", f=NQ * FOUT),
                          in_ap=table[:, :],
                          idxs_ap=idx_all[:, icol + c0 * 8:icol + c1 * 8],
                          num_idxs=(c1 - c0) * P,
                          num_idxs_reg=(c1 - c0) * P,
                          elem_size=NQ * FOUT,
                          single_packet=SP,
                          queue_num=q,
                      )
                      if len(gather_insts[q]) >= QD:
                          _add_dep_helper(gi.ins, gather_insts[q][-QD],
                                          sync=True,
                                          reason="swdge ring throttle")
                      gather_insts[q].append(gi.ins)

              if not do_compute:
                  continue

              for bk in blks:
                  po = popool.tile([P, FOUT], f32, tag="po")
                  started = False
                  for (gcol, ql) in st["blk_chunks"][bk]:
                      j = gcol - c0g
                      g0 = j * NQ * FOUT
                      for g in ql:
                          lc = int(st["scol"][(gcol, g)]) - sc0
                          nc.tensor.matmul(
                              out=po[:], lhsT=s_sb[:, lc:lc + P],
                              rhs=gt[:, g0 + g * FOUT:g0 + (g + 1) * FOUT],
                              start=not started, stop=False,
                          )
                          started = True
                  nc.tensor.matmul(
                      out=po[:], lhsT=ones_t[:], rhs=brep_t[:],
                      start=not started, stop=True,
                  )
                  ob = opool.tile([P, FOUT], f32, tag="ob")
                  nc.scalar.activation(
                      out=ob[:], in_=po[:],
                      func=mybir.ActivationFunctionType.Relu)
                  nc.sync.dma_start(
                      out=out_sh[bk * P:(bk + 1) * P, :], in_=ob[:])

    nc.compile()
    _split_excess_waits(nc)
    return nc


def _prepare(X, edge_row, edge_col, edge_vals, W, b, balance=True, cfg=None,
             variant="full"):
    """Build the compiled Bass program + per-core input maps."""
    X = np.asarray(X, dtype=np.float32)
    edge_row = np.asarray(edge_row, dtype=np.int64)
    edge_col = np.asarray(edge_col, dtype=np.int64)
    edge_vals = np.asarray(edge_vals, dtype=np.float32)
    W = np.asarray(W, dtype=np.float32)
    b = np.asarray(b, dtype=np.float32)

    if balance:
        perm = _balance_perm(edge_row)
        inv = np.empty(N, dtype=np.int64)
        inv[perm] = np.arange(N)
        X = X[perm]
        edge_row = inv[edge_row]
        edge_col = inv[edge_col]
    else:
        perm = None

    struct, idx_np, s_np = _build_plan(edge_row, edge_col, edge_vals)
    nc = _build_nc(struct, variant=variant, cfg=cfg)

    b_rep = np.tile(b[None, :], (P, 1)).astype(bfloat16)
    in_maps = []
    for c in range(NCORES):
        xt_pad = np.zeros((FIN, SHARD_PAD), dtype=bfloat16)
        xt_pad[:, :SHARD] = X[c * SHARD:(c + 1) * SHARD].T.astype(bfloat16)
        in_maps.append({
            "xt_sh": xt_pad, "w_in": W.astype(bfloat16), "b_rep": b_rep,
            "idx_in": idx_np[c], "s_in": s_np[c],
        })
    return nc, in_maps, perm, struct


def _assemble(results, perm=None):
    out = np.concatenate(
        [results[c]["out_sh"][:SHARD] for c in range(NCORES)], axis=0)
    if perm is not None:
        unperm = np.empty_like(out)
        unperm[perm] = out
        out = unperm
    return out


def kernel(X, edge_row, edge_col, edge_vals, W, b):
    nc, in_maps, perm, _ = _prepare(X, edge_row, edge_col, edge_vals, W, b)
    res = run_bass_kernel_spmd(nc, in_maps, core_ids=list(range(NCORES)))
    return _assemble(res.results, perm)


# revision 4
# speedup vs baseline: 2.3841x; 1.1002x over previous
"""GCN layer (X@W -> edge gather/scale -> segment-sum by dest -> +b -> relu)
as a Bass/Tile kernel on 8 Trainium2 NeuronCores — v3 (quad table + host-S).

Strategy (1D node partition, SPMD single program):
  - Nodes relabeled on host (degree-balanced snake deal) so every
    (core, 128-dest block) holds ~equal edge counts -> chunk padding is
    near-zero.  Nodes sharded 12500/core.
  - Each core computes its XW shard with bf16 PE matmuls; an AllGather
    replicates the full XW table as bf16 QUAD rows: table row q holds
    nodes 4q..4q+3 = 256 bf16 = 512B.  512B SWDGE elements avoid the
    <512B read-modify-write DMA penalty, and 25088 quad rows fit one
    int16 index window (no window split, halving chunk padding).
  - Edges partitioned by dest core, bucketed by (dest blk, src qpos),
    qpos groups packed back-to-back per core.  Each 128-edge chunk
    gathers its 128 source quad rows via dma_gather on 4 SWDGE queues.
  - The selection matrix S[e, d] = val[e] * (dest[e]==d) is built ON THE
    HOST (bf16) and streamed from DRAM - measurement shows DVE/Act ops
    serialize with SWDGE gather traffic on this hardware, so the S build
    must not touch the engines.  Per chunk, one PE matmul per source
    qpos present accumulates psum[128 dests, 64] += S_q^T @ G[:, q].
    A ones/128 x bias matmul folds +b into PSUM; relu on eviction.

All chunk counts are padded to the max over cores so all 8 cores run the
same program (required for the collective / PJRT SPMD launch).
"""

import math
from contextlib import ExitStack

import numpy as np
from ml_dtypes import bfloat16

import concourse.bacc as bacc
import concourse.mybir as mybir
import concourse.tile as tile
from concourse.bass import _add_dep_helper
from concourse.bass_utils import run_bass_kernel_spmd

# Problem constants (hardcoded per contract; kernel must be self-contained).
N = 100000
E = 1600000
FIN = 256
FOUT = 64
NCORES = 8

P = 128                       # partitions / block size
SHARD = N // NCORES           # 12500 dest nodes per core
NBLK = math.ceil(SHARD / P)   # 98 dest blocks per core
SHARD_PAD = NBLK * P          # 12544 (X zero-padded rows)
NQ = 4                        # nodes per table quad row
QUADS = SHARD_PAD // NQ       # 3136 quad rows per core
TABLE_QUADS = NCORES * QUADS  # 25088 quad rows (< 32768: one int16 window)
SB_BLOCKS = 2                 # dest blocks per super-batch
NSB = math.ceil(NBLK / SB_BLOCKS)


def set_sb(n):
    """Reconfigure super-batch width (affects plan + program)."""
    global SB_BLOCKS, NSB
    SB_BLOCKS = n
    NSB = math.ceil(NBLK / SB_BLOCKS)
KH = FIN // P                 # 2 contraction halves in the GEMM
NQUEUES = 4                   # SWDGE queues for gathers (ucode max 4)
QDEPTH = 64                   # in-flight gather parts per queue before a
                              # throttle dep (33 parts/queue in practice, so
                              # effectively unthrottled; SWDGE ring holds 128)
GMAXCH = 12                   # max chunks per gather part


def _balance_perm(edge_row):
    """Relabel nodes so per-(core, blk) dest-edge counts are balanced.

    Snake-deals nodes in degree-desc order into the 784 (core, blk) bins
    (mirroring the id->(core,blk) map incl. the 84-slot tail block per
    core).  Returns perm_old_of_new [N]: new id -> old node."""
    deg = np.bincount(edge_row, minlength=N).astype(np.int64)
    order = np.argsort(-deg, kind="stable")
    nbins = NCORES * NBLK
    cap = np.full(nbins, P, dtype=np.int64)
    cap[NBLK - 1::NBLK] = SHARD - (NBLK - 1) * P
    bin_base = (np.arange(nbins) // NBLK) * SHARD + (np.arange(nbins) % NBLK) * P
    perm = np.empty(N, dtype=np.int64)
    fill = np.zeros(nbins, dtype=np.int64)
    pos = 0
    fwd = True
    bins_seq = np.arange(nbins)
    while pos < N:
        avail = bins_seq[fill < cap]
        if not fwd:
            avail = avail[::-1]
        take = min(len(avail), N - pos)
        sel = avail[:take]
        perm[bin_base[sel] + fill[sel]] = order[pos:pos + take]
        fill[sel] += 1
        pos += take
        fwd = not fwd
    return perm


def _build_plan(edge_row, edge_col, edge_vals):
    """Host-side edge partition/sort/pad.

    Edges are bucketed (core, blk) with qpos groups packed back-to-back
    per core; chunk capacity per blk = ceil(max-core count / 128).  Per
    chunk the UNION over cores of source qpos groups present decides the
    matmul list; absent groups simply have all-zero S columns."""
    core = edge_row // SHARD
    r_local = edge_row - core * SHARD
    blk = r_local // P
    dst = (r_local - blk * P).astype(np.int64)
    src_core = edge_col // SHARD
    src_local = edge_col - src_core * SHARD
    qpos = src_local % NQ
    quad = src_core * QUADS + src_local // NQ     # < 25088, int16-safe

    order = np.lexsort((qpos, blk, core))
    core_s = core[order]
    blk_s = blk[order]
    qpos_s = qpos[order]
    quad_s = quad[order]
    dst_s = dst[order]
    val_s = edge_vals[order].astype(np.float32)

    # counts per (core, blk, qpos) and per (core, blk)
    key_cbq = (core_s * NBLK + blk_s) * NQ + qpos_s
    counts_cbq = np.bincount(key_cbq, minlength=NCORES * NBLK * NQ).reshape(
        NCORES, NBLK, NQ)
    counts_cb = counts_cbq.sum(axis=2)
    pfx_cbq = np.zeros((NCORES, NBLK, NQ + 1), dtype=np.int64)
    np.cumsum(counts_cbq, axis=2, out=pfx_cbq[:, :, 1:])

    # per-blk chunk capacity = ceil(max-core total / 128)
    nch_b = np.ceil(counts_cb.max(axis=0) / P).astype(np.int64)  # [NBLK]
    assert nch_b.min() >= 1
    CTOT = int(nch_b.sum())
    ITOT = CTOT * (P // 16)

    sb_of_blk = np.arange(NBLK) // SB_BLOCKS
    # global chunk column of each blk's first chunk (sb-major walk order)
    blk_c0 = np.zeros(NBLK, dtype=np.int64)
    cur = 0
    for sb in range(NSB):
        for bk in np.where(sb_of_blk == sb)[0]:
            blk_c0[bk] = cur
            cur += nch_b[bk]
    assert cur == CTOT
    sb_len = np.zeros(NSB, dtype=np.int64)     # chunks per sb
    sb_c0 = np.zeros(NSB + 1, dtype=np.int64)  # first chunk col of sb
    for sb in range(NSB):
        bks = np.where(sb_of_blk == sb)[0]
        sb_c0[sb] = blk_c0[bks[0]]
        sb_len[sb] = nch_b[bks].sum()
    sb_c0[NSB] = CTOT

    # per-chunk qpos union over cores; blk_chunks[bk] = [(gcol, qlist)]
    blk_chunks = []
    for bk in range(NBLK):
        lst = []
        for k in range(int(nch_b[bk])):
            lo, hi = k * P, (k + 1) * P
            ql = []
            for g in range(NQ):
                lo_g = pfx_cbq[:, bk, g]
                hi_g = pfx_cbq[:, bk, g + 1]
                if np.any((lo_g < hi) & (hi_g > lo)):
                    ql.append(g)
            lst.append((int(blk_c0[bk]) + k, ql))
        blk_chunks.append(lst)

    # S column layout: walk sb -> blk -> chunk -> qpos in qlist
    scol = {}
    s_col0 = np.zeros(NSB + 1, dtype=np.int64)
    cur = 0
    for sb in range(NSB):
        s_col0[sb] = cur
        for bk in np.where(sb_of_blk == sb)[0]:
            for (gcol, ql) in blk_chunks[bk]:
                for g in ql:
                    scol[(gcol, g)] = cur
                    cur += P
    s_col0[NSB] = cur
    STOT = cur

    # per-edge global slot: qpos groups packed back-to-back per (core, blk)
    first_cbq = np.zeros(NCORES * NBLK * NQ + 1, dtype=np.int64)
    np.cumsum(counts_cbq.reshape(-1), out=first_cbq[1:])
    rank = np.arange(len(core_s)) - first_cbq[key_cbq]
    within = pfx_cbq[core_s, blk_s, qpos_s] + rank
    slot = blk_c0[blk_s] * P + within
    gcol_e = slot // P
    eslot_e = slot - gcol_e * P

    # staged arrays: idx [128, ITOT] int16 wrapped per sb; S [128, STOT] bf16
    idx_streams = np.zeros((NCORES, CTOT * P), dtype=np.int16)
    for c in range(NCORES):
        m = core_s == c
        idx_streams[c, slot[m]] = quad_s[m].astype(np.int16)
    idx_np = np.zeros((NCORES, P, ITOT), dtype=np.int16)
    for sb in range(NSB):
        L = int(sb_len[sb]) * P
        if L == 0:
            continue
        s0 = int(sb_c0[sb]) * P
        i0 = int(sb_c0[sb]) * (P // 16)
        seg = idx_streams[:, s0:s0 + L].reshape(NCORES, L // 16, 16)
        seg = seg.transpose(0, 2, 1)
        idx_np[:, :, i0:i0 + L // 16] = np.tile(seg, (1, 8, 1))

    scol_arr = np.full((CTOT, NQ), -1, dtype=np.int64)
    for (gcol, g), c0 in scol.items():
        scol_arr[gcol, g] = c0
    col_e = scol_arr[gcol_e, qpos_s] + dst_s
    assert (scol_arr[gcol_e, qpos_s] >= 0).all()
    s_np = np.zeros((NCORES, P, STOT), dtype=bfloat16)
    for c in range(NCORES):
        m = core_s == c
        s_np[c, eslot_e[m], col_e[m]] = val_s[m].astype(bfloat16)

    struct = dict(
        nch_b=nch_b, sb_of_blk=sb_of_blk, blk_c0=blk_c0, sb_len=sb_len,
        sb_c0=sb_c0, blk_chunks=blk_chunks, s_col0=s_col0, scol=scol,
        CTOT=CTOT, ITOT=ITOT, STOT=STOT,
    )
    return struct, idx_np, s_np


_NO_SPLIT = ("InstEventSemaphore", "InstDrain", "InstCollectiveCompute",
             "InstCall", "InstUnconditionalBranch", "InstConditionalBranch")


def _split_excess_waits(nc):
    """Deterministic post-pass: TRN2 instructions tolerate very few sync
    waits (walrus rejects with 'Too many sync wait commands').  Move all
    but one semaphore wait of every ordinary instruction onto wait-only
    InstEventSemaphore instructions inserted just before it on the same
    engine."""
    for blk in nc.main_func.blocks:
        out = []
        for ins in blk.instructions:
            si = ins.sync_info
            tn = type(ins).__name__
            if si is None or tn in _NO_SPLIT or len(si.on_wait) <= 1:
                out.append(ins)
                continue
            waits = list(si.on_wait)
            keep, excess = waits[:1], waits[1:]
            while excess:
                batch, excess = excess[:2], excess[2:]
                ev = mybir.InstEventSemaphore(
                    name=nc.get_next_instruction_name(), ins=[], outs=[])
                ev.engine = ins.engine
                ev.sync_info = mybir.SyncInfo(on_wait=batch, on_update=[])
                out.append(ev)
            ins.sync_info = mybir.SyncInfo(
                on_wait=keep, on_update=list(si.on_update))
            out.append(ins)
        blk.instructions[:] = out


def _build_nc(struct, variant="full", cfg=None):
    # variant: "full" | "go" (gathers only) | "hsgs" (gathers + S stream)
    #          | "p1" (GEMM) | "p12" (GEMM+AG) | "nop"
    # "xN" suffix repeats the body N times inside one NEFF (timing).
    cfg = {**dict(qdepth=QDEPTH, gmaxch=GMAXCH, single_packet=False),
           **(cfg or {})}
    reps = 1
    if " x" in variant:
        variant, reps_s = variant.split(" x")
        reps = int(reps_s)
    st = struct
    CTOT, ITOT, STOT = st["CTOT"], st["ITOT"], st["STOT"]
    nc = bacc.Bacc("TRN2", target_bir_lowering=False, debug=False,
                   num_devices=NCORES, num_swdge_queues=NQUEUES)
    f32 = mybir.dt.float32
    bf16 = mybir.dt.bfloat16
    i16 = mybir.dt.int16

    xt_sh = nc.dram_tensor("xt_sh", [FIN, SHARD_PAD], bf16, kind="ExternalInput")
    w_in = nc.dram_tensor("w_in", [FIN, FOUT], bf16, kind="ExternalInput")
    b_rep = nc.dram_tensor("b_rep", [P, FOUT], bf16, kind="ExternalInput")
    idx_in = nc.dram_tensor("idx_in", [P, ITOT], i16, kind="ExternalInput")
    s_in = nc.dram_tensor("s_in", [P, STOT], bf16, kind="ExternalInput")

    xw_sh = nc.dram_tensor("xw_sh", [SHARD_PAD, FOUT], bf16, kind="Internal")
    table = nc.dram_tensor("table", [TABLE_QUADS, NQ * FOUT], bf16,
                           kind="Internal", addr_space="Shared")
    out_sh = nc.dram_tensor("out_sh", [SHARD_PAD, FOUT], f32,
                            kind="ExternalOutput")

    do_gemm = variant in ("full", "p1", "p12", "p12g")
    do_ag = variant in ("full", "p12", "p12g", "ag")
    do_gather = variant in ("full", "go", "hsgs", "p12g")
    do_stream = variant in ("full", "hsgs")
    do_compute = variant == "full"

    with tile.TileContext(nc) as tc, ExitStack() as ctx:
        consts = ctx.enter_context(tc.tile_pool(name="consts", bufs=1))
        gpool = ctx.enter_context(tc.tile_pool(name="gpool", bufs=5))
        hspool = ctx.enter_context(tc.tile_pool(name="hspool", bufs=3))
        opool = ctx.enter_context(tc.tile_pool(name="opool", bufs=4))
        xpool = ctx.enter_context(tc.tile_pool(name="xpool", bufs=3))
        pmpool = ctx.enter_context(
            tc.tile_pool(name="pmpool", bufs=2, space="PSUM"))
        popool = ctx.enter_context(
            tc.tile_pool(name="popool", bufs=6, space="PSUM"))

        brep_t = consts.tile([P, FOUT], bf16)
        nc.sync.dma_start(out=brep_t[:], in_=b_rep[:])
        if do_compute:
            ones_t = consts.tile([P, P], bf16, tag="ones")
            nc.vector.memset(ones_t[:], 1.0 / P)
        if do_gather:
            idx_all = consts.tile([P, ITOT], i16, tag="idxall")
            nc.sync.dma_start(out=idx_all[:], in_=idx_in[:])
        if do_gemm:
            w_t = []
            for h in range(KH):
                wt = consts.tile([P, FOUT], bf16, tag=f"w{h}")
                nc.sync.dma_start(out=wt[:], in_=w_in[h * P:(h + 1) * P, :])
                w_t.append(wt)

        QD, GM, SP = cfg["qdepth"], cfg["gmaxch"], cfg["single_packet"]
        gather_insts = {q: [] for q in range(NQUEUES)}
        gq = [0]

        for rep in range(reps):
          # ---------------- phase 1: GEMM shard ----------------
          GRP = 896 if SHARD_PAD % 896 == 0 else SHARD_PAD
          for g in (range(SHARD_PAD // GRP) if do_gemm else ()):
              xts = []
              for h in range(KH):
                  xt = xpool.tile([P, GRP], bf16, tag=f"xt{h}")
                  nc.sync.dma_start(
                      out=xt[:], in_=xt_sh[h * P:(h + 1) * P,
                                           g * GRP:(g + 1) * GRP])
                  xts.append(xt)
              for c in range(GRP // P):
                  bk = g * (GRP // P) + c
                  mm = pmpool.tile([P, FOUT], f32, tag="mm")
                  for h in range(KH):
                      nc.tensor.matmul(
                          out=mm[:], lhsT=xts[h][:, c * P:(c + 1) * P],
                          rhs=w_t[h][:], start=(h == 0), stop=(h == KH - 1))
                  om = opool.tile([P, FOUT], bf16, tag="om")
                  nc.vector.tensor_copy(out=om[:], in_=mm[:])
                  nc.sync.dma_start(
                      out=xw_sh[bk * P:(bk + 1) * P, :], in_=om[:])

          # ---------------- phase 2: AllGather the XW quad table ------------
          if do_ag:
              nc.gpsimd.collective_compute(
                  kind="AllGather", op=mybir.AluOpType.bypass,
                  replica_groups=[list(range(NCORES))],
                  ins=[xw_sh[:]], outs=[table[:]],
              )

          # ---------------- phase 3: gather + segment-sum ----------------
          if not do_compute:
              ob = opool.tile([P, FOUT], f32, tag="ob")
              obb = opool.tile([P, FOUT], bf16, tag="obb")
              if do_ag:
                  nc.sync.dma_start(out=obb[:], in_=table[:P, :FOUT])
              elif do_gemm:
                  nc.sync.dma_start(out=obb[:], in_=xw_sh[:P, :FOUT])
              else:
                  nc.sync.dma_start(out=obb[:], in_=b_rep[:])
              nc.vector.tensor_copy(out=ob[:], in_=obb[:])
              nc.sync.dma_start(out=out_sh[:P, :], in_=ob[:])

          sb_of_blk = st["sb_of_blk"]
          for sb in (range(NSB) if (do_gather or do_stream) else ()):
              blks = [bk for bk in range(NBLK) if sb_of_blk[bk] == sb]
              nch = int(st["sb_len"][sb])
              c0g = int(st["sb_c0"][sb])

              if do_stream:
                  sc0 = int(st["s_col0"][sb])
                  sc1 = int(st["s_col0"][sb + 1])
                  s_sb = hspool.tile([P, sc1 - sc0], bf16, tag="ssb")
                  nc.sync.dma_start(out=s_sb[:], in_=s_in[:, sc0:sc1])

              gt = None
              if do_gather:
                  icol = c0g * (P // 16)
                  gt = gpool.tile([P, nch * NQ * FOUT], bf16, tag="g")
                  nparts = max(1, math.ceil(nch / GM))
                  psz = math.ceil(nch / nparts)
                  for c0 in range(0, nch, psz):
                      c1 = min(c0 + psz, nch)
                      q = gq[0] % NQUEUES
                      gq[0] += 1
                      gi = nc.gpsimd.dma_gather(
                          out_ap=gt[:, c0 * NQ * FOUT:c1 * NQ * FOUT].rearrange(
                              "p (c f) -> p c f", f=NQ * FOUT),
                          in_ap=table[:, :],
                          idxs_ap=idx_all[:, icol + c0 * 8:icol + c1 * 8],
                          num_idxs=(c1 - c0) * P,
                          num_idxs_reg=(c1 - c0) * P,
                          elem_size=NQ * FOUT,
                          single_packet=SP,
                          queue_num=q,
                      )
                      if len(gather_insts[q]) >= QD:
                          _add_dep_helper(gi.ins, gather_insts[q][-QD],
                                          sync=True,
                                          reason="swdge ring throttle")
                      gather_insts[q].append(gi.ins)

              if not do_compute:
                  continue

              for bk in blks:
                  po = popool.tile([P, FOUT], f32, tag="po")
                  started = False
                  for (gcol, ql) in st["blk_chunks"][bk]:
                      j = gcol - c0g
                      g0 = j * NQ * FOUT
                      for g in ql:
                          lc = int(st["scol"][(gcol, g)]) - sc0
                          nc.tensor.matmul(
                              out=po[:], lhsT=s_sb[:, lc:lc + P],
                              rhs=gt[:, g0 + g * FOUT:g0 + (g + 1) * FOUT],
                              start=not started, stop=False,
                          )
                          started = True
                  nc.tensor.matmul(
                      out=po[:], lhsT=ones_t[:], rhs=brep_t[:],
                      start=not started, stop=True,
                  )
                  ob = opool.tile([P, FOUT], f32, tag="ob")
                  nc.scalar.activation(
                      out=ob[:], in_=po[:],
                      func=mybir.ActivationFunctionType.Relu)
                  nc.sync.dma_start(
                      out=out_sh[bk * P:(bk + 1) * P, :], in_=ob[:])

    nc.compile()
    _split_excess_waits(nc)
    return nc


def _prepare(X, edge_row, edge_col, edge_vals, W, b, balance=True, cfg=None,
             variant="full"):
    """Build the compiled Bass program + per-core input maps."""
    X = np.asarray(X, dtype=np.float32)
    edge_row = np.asarray(edge_row, dtype=np.int64)
    edge_col = np.asarray(edge_col, dtype=np.int64)
    edge_vals = np.asarray(edge_vals, dtype=np.float32)
    W = np.asarray(W, dtype=np.float32)
    b = np.asarray(b, dtype=np.float32)

    if balance:
        perm = _balance_perm(edge_row)
        inv = np.empty(N, dtype=np.int64)
        inv[perm] = np.arange(N)
        X = X[perm]
        edge_row = inv[edge_row]
        edge_col = inv[edge_col]
    else:
        perm = None

    struct, idx_np, s_np = _build_plan(edge_row, edge_col, edge_vals)
    nc = _build_nc(struct, variant=variant, cfg=cfg)

    b_rep = np.tile(b[None, :], (P, 1)).astype(bfloat16)
    in_maps = []
    for c in range(NCORES):
        xt_pad = np.zeros((FIN, SHARD_PAD), dtype=bfloat16)
        xt_pad[:, :SHARD] = X[c * SHARD:(c + 1) * SHARD].T.astype(bfloat16)
        in_maps.append({
            "xt_sh": xt_pad, "w_in": W.astype(bfloat16), "b_rep": b_rep,
            "idx_in": idx_np[c], "s_in": s_np[c],
        })
    return nc, in_maps, perm, struct


def _assemble(results, perm=None):
    out = np.concatenate(
        [results[c]["out_sh"][:SHARD] for c in range(NCORES)], axis=0)
    if perm is not None:
        unperm = np.empty_like(out)
        unperm[perm] = out
        out = unperm
    return out


def kernel(X, edge_row, edge_col, edge_vals, W, b):
    nc, in_maps, perm, _ = _prepare(X, edge_row, edge_col, edge_vals, W, b)
    res = run_bass_kernel_spmd(nc, in_maps, core_ids=list(range(NCORES)))
    return _assemble(res.results, perm)
